# revision 17
# baseline (speedup 1.0000x reference)
"""Trainium2 Bass kernel for nn_Attention_5299989643989.

GQA attention forward (B=2, T=2048, C=1024, 16 q heads / 4 kv heads, D=64)
with value-embedding gating, rotary embedding, qk rms-norm, causal softmax.

Sharding: 8 cores = batch (2) x kv-head-group (4).  Each core computes its
4 q heads / 1 kv head end-to-end plus the Wo row-shard partial output; the
host sums the 4 partials per batch (the Wo all-reduce, done at unshard).

bf16 datapath: inputs are packed host-side into partition-major bf16 arrays
so each resident load is one large DMA; all matmuls run in bf16 (1 cycle/row
on the PE at any width); the output is stored bf16 and the host accumulates
partials in f32.

Schedule notes (cost-model driven):
 - The PE p-states reward continuous execution, so projections run 2-3
   query-groups ahead of the attention loop and the first group's
   projection runs kc-outer across four open PSUM banks so matmuls start
   as soon as the first interleaved weight/x DMA slice lands.
 - The causal mask is applied on the PE: diagonal score tiles accumulate a
   -1e9 strict-lower-triangular constant before the exp, so no vector
   engine sits between the exp and the AV matmul.
 - exp is batched two PSUM banks per activation call ([P, 2, 512-lo] APs)
   to amortize the ACT access latency.
 - GPSIMD has no PSUM port: all PSUM evacuation is on DVE (pjg, qT, kT2,
   denominators, yT, half the output tiles) and ACT (other half of the
   output tiles); Pool does SBUF-only work (rsqrt Newton chain, ve-gating,
   reciprocal->bf16 copies, constants).
"""

import numpy as np

import concourse.bacc as bacc
import concourse.bass as bass
import concourse.tile as tile
from concourse import mybir
from concourse.masks import make_identity

f32 = mybir.dt.float32
bf16 = mybir.dt.bfloat16
AF = mybir.ActivationFunctionType

B, T, C = 2, 2048, 1024
N_HEAD, N_KV_HEAD, D = 16, 4, 64
HQ = N_HEAD // N_KV_HEAD  # q heads per core = 4
P = 128
NT = T // P       # 16 token chunks
KC = C // P       # 8 contraction chunks
IB = 512          # query block
NBI = T // IB     # 4 query blocks
GRP = IB // P     # 4 token chunks per query block
SC = 1.2 * 1.2 / 8.0  # folded qk scale: rms 1.2 factors * 1/sqrt(64)
H32 = D // 2
WJ = 385          # jammed projection width: q 256 | k 64 | v 64 | gate 1

# Schraudolph exp on DVE (bf16 bit pattern via int16 convert):
#   exp(SC*s) ~= bitcast_bf16(int16(EXP_A*s + EXP_B))
# EXP_A = SC * 2^7/ln2; EXP_B = 127*2^7 - 2^7*C with C ~= 0.0579 ln-units
# (minimum-RMS magic constant, Schraudolph 1999).  Only used on
# off-diagonal score tiles where |s| <= 64 so the int16 never saturates.
EXP_A = SC * 184.66496  # SC * 2^7 / ln 2
EXP_B = 16248.58        # (127*2^23 - 486411) / 2^16, min-RMS Schraudolph
EXP_SPLIT = 3     # every EXP_SPLIT'th off-diagonal exp pair runs on DVE
ROPE_POOL = True  # rope+normalize for groups >= 1 run on Pool


def build_program():
    nc = bacc.Bacc("TRN2", target_bir_lowering=False, debug=False, num_devices=8)

    xh = nc.dram_tensor("xh", [P, NBI, KC, IB], bf16, kind="ExternalInput")
    wrh = nc.dram_tensor("wrh", [P, KC, WJ], bf16, kind="ExternalInput")
    csvh = nc.dram_tensor("csvh", [P, NT * 128], bf16, kind="ExternalInput")
    woh = nc.dram_tensor("woh", [P, 2, C], bf16, kind="ExternalInput")
    out = nc.dram_tensor("out", [T, C], bf16, kind="ExternalOutput")

    with tile.TileContext(nc) as tc:
        with (
            tc.tile_pool(name="consts", bufs=1) as consts,
            tc.tile_pool(name="resid", bufs=1) as resid,
            tc.tile_pool(name="xload", bufs=2) as xload,
            tc.tile_pool(name="rot", bufs=2) as rot,
            tc.tile_pool(name="small", bufs=4) as small,
            tc.tile_pool(name="exps", bufs=4) as exps,
            tc.tile_pool(name="outsb", bufs=2) as outsb,
            tc.tile_pool(name="psmm", bufs=2, space="PSUM") as psmm,
            tc.tile_pool(name="pssc", bufs=2, space="PSUM") as pssc,
            tc.tile_pool(name="psy", bufs=2, space="PSUM") as psy,
        ):
            # ---- group-0 resident loads, kc-pair interleaved so the first
            # projection matmuls can start as soon as slice 0 lands ----
            wr_sb = consts.tile([P, KC, WJ], bf16)
            xt0 = xload.tile([P, KC, IB], bf16, name="xt0", tag="xt")
            for pr in range(4):
                k0, k1 = 2 * pr, 2 * pr + 2
                nc.sync.dma_start(wr_sb[:, k0:k1, :], wrh[:, k0:k1, :])
                nc.sync.dma_start(xt0[:, k0:k1, :], xh[:, 0, k0:k1, :])
            # csv = cos(32) | sin(32) | 3*ve (64) per token
            csv_sb = consts.tile([P, NT, 128], bf16)
            nc.sync.dma_start(
                csv_sb[:], csvh[:].rearrange("p (n d) -> p n d", d=128)
            )
            ident = consts.tile([P, P], bf16)
            make_identity(nc, ident[:])
            # strict-lower-triangular -1e9 block: added into diagonal score
            # tiles (PSUM accumulate) so exp() itself applies the causal mask
            mneg = consts.tile([P, P], bf16)
            nc.gpsimd.memset(mneg[:], -1.0e9)
            nc.gpsimd.affine_select(
                out=mneg[:],
                in_=mneg[:],
                compare_op=mybir.AluOpType.is_gt,
                fill=0.0,
                base=0,
                pattern=[[-1, P]],
                channel_multiplier=1,
            )
            # selector for the denominator broadcast: rows 0/32 of rr ->
            # partition halves 0:64 / 64:128 of the outer-product result
            sel = consts.tile([33, P], bf16)
            nc.gpsimd.memset(sel[:], 0.0)
            nc.gpsimd.memset(sel[0:1, 0:64], 1.0)
            nc.gpsimd.memset(sel[32:33, 64:128], 1.0)
            zero_sb = consts.tile([P, 1], f32)
            nc.vector.memset(zero_sb[:], 0.0)
            rsq_k = consts.tile([P, 1], mybir.dt.uint32)
            nc.vector.memset(rsq_k[:], 0x5F3759DF)

            # ---- residents written by the kernel ----
            qT = resid.tile([P, 2, T], bf16)   # [h0|h1] rows, [h2|h3] rows
            kT2 = resid.tile([P, T], bf16)     # kT duplicated in both halves
            v_aug = resid.tile([P, NT, D + 1], bf16)  # v plus ones column
            nc.gpsimd.memset(v_aug[:, :, D : D + 1], 1.0)
            yT1 = resid.tile([P, T], bf16)        # yT heads 0,1
            yT2 = resid.tile([P, T], bf16)        # yT heads 2,3
            rs1 = resid.tile([33, T], f32)   # denominators: h0 row0, h1 row32
            rs2 = resid.tile([33, T], f32)
            nc.gpsimd.memset(rs1[:], 1.0)
            nc.gpsimd.memset(rs2[:], 1.0)
            rc1 = resid.tile([33, T], f32)
            rc2 = resid.tile([33, T], f32)
            rr1 = resid.tile([33, T], bf16)   # bf16 copies for the PE
            rr2 = resid.tile([33, T], bf16)

            wo_sb = consts.tile([P, 2, C], bf16)

            def load_x(bi):
                xt = xload.tile([P, KC, IB], bf16, name=f"xt{bi}", tag="xt")
                nc.sync.dma_start(xt[:], xh[:, bi])
                return xt

            pjgs = {}

            def proj(bi, xt, kc_outer=False):
                """Jammed q|k|v|gate projection for one 512-token group."""
                pjg = rot.tile([P, GRP, WJ], bf16, tag="pjg", bufs=3)
                pjgs[bi] = pjg
                if kc_outer:
                    # four PSUM banks open at once (borrow the score pool),
                    # contraction outermost to chase the interleaved DMAs
                    pj_aps = []
                    for tl in range(GRP):
                        if tl < 2:
                            pjt = psmm.tile([P, 512], f32, tag="mm", name=f"pjt{tl}")
                            pj_aps.append(pjt[:, 0:WJ])
                        else:
                            pjt = pssc.tile(
                                [P, 2, 512], f32, tag="sc", name=f"pjt{tl}"
                            )
                            pj_aps.append(pjt[:, 0, 0:WJ])
                    for kc in range(KC):
                        for tl in range(GRP):
                            nc.tensor.matmul(
                                pj_aps[tl],
                                xt[:, kc, tl * P : (tl + 1) * P],
                                wr_sb[:, kc, :],
                                start=(kc == 0),
                                stop=(kc == KC - 1),
                            )
                    for tl in range(GRP):
                        nc.vector.tensor_copy(pjg[:, tl, :], pj_aps[tl])
                else:
                    for tl in range(GRP):
                        pj = psmm.tile([P, 512], f32, tag="mm")
                        for kc in range(KC):
                            nc.tensor.matmul(
                                pj[:, 0:WJ],
                                xt[:, kc, tl * P : (tl + 1) * P],
                                wr_sb[:, kc, :],
                                start=(kc == 0),
                                stop=(kc == KC - 1),
                            )
                        nc.vector.tensor_copy(pjg[:, tl, :], pj[:, 0:WJ])
                # gate tanh for the whole group in one ACT call
                tgg = small.tile([P, GRP], f32, tag="tgg")
                nc.scalar.activation(
                    tgg[:],
                    pjg[:, :, 384:385].rearrange("p g o -> p (g o)"),
                    AF.Tanh,
                    scale=0.5,
                    bias=zero_sb[:],
                )
                proj.tgg = tgg

            def rope(bi, tgg, halves=2):
                """rope + rms-norm + rstd + normalize + ve-gating for group bi."""
                pjg = pjgs[bi]
                qkr = rot.tile([P, GRP, 320], bf16, tag="qkr", bufs=2)
                tmp = rot.tile([P, GRP, 160], bf16, tag="tmp", bufs=2)
                sqg = rot.tile([P, GRP, 320], bf16, tag="sqg", bufs=2)
                msg = small.tile([P, GRP * 5], f32, tag="msg")
                rstdg = small.tile([P, GRP * 5], f32, tag="rstdg")
                nwt = small.tile([P, GRP * 5], f32, tag="nwt")
                qkn = rot.tile([P, GRP, 320], bf16, tag="qkn", bufs=2)
                ve_ = nc.gpsimd if (ROPE_POOL and bi > 0) else nc.vector
                gstep = GRP // halves
                for hf in range(halves):
                    g0, g1_ = gstep * hf, gstep * (hf + 1)
                    f0, f1 = 5 * gstep * hf, 5 * gstep * (hf + 1)
                    nf = f1 - f0
                    qv5 = pjg[:, g0:g1_, 0:320].rearrange(
                        "p g (h d) -> p g h d", d=D
                    )
                    ro5 = qkr[:, g0:g1_, :].rearrange("p g (h d) -> p g h d", d=D)
                    t5 = tmp[:, g0:g1_, :].rearrange("p g (h d) -> p g h d", d=H32)
                    cs = csv_sb[:, bi * GRP + g0 : bi * GRP + g1_, 0:32]
                    sn = csv_sb[:, bi * GRP + g0 : bi * GRP + g1_, 32:64]
                    cos5 = cs.unsqueeze(2).broadcast_to([P, g1_ - g0, 5, H32])
                    sin5 = sn.unsqueeze(2).broadcast_to([P, g1_ - g0, 5, H32])
                    q1 = qv5[:, :, :, 0:H32]
                    q2 = qv5[:, :, :, H32:D]
                    ve_.tensor_mul(ro5[:, :, :, 0:H32], q1, cos5)
                    ve_.tensor_mul(t5[:], q2, sin5)
                    ve_.tensor_add(
                        ro5[:, :, :, 0:H32], ro5[:, :, :, 0:H32], t5[:]
                    )
                    ve_.tensor_mul(ro5[:, :, :, H32:D], q2, cos5)
                    ve_.tensor_mul(t5[:], q1, sin5)
                    ve_.tensor_sub(
                        ro5[:, :, :, H32:D], ro5[:, :, :, H32:D], t5[:]
                    )

                    nc.vector.tensor_mul(
                        sqg[:, g0:g1_, :], qkr[:, g0:g1_, :], qkr[:, g0:g1_, :]
                    )
                    nc.vector.reduce_sum(
                        msg[:, f0:f1],
                        sqg[:, g0:g1_, :].rearrange("p g (h d) -> p (g h) d", d=D),
                        axis=mybir.AxisListType.X,
                    )
                    # m = mean + eps; rstd = m^-1/2 by bit-trick seed + two
                    # Newton iterations (f32 bit layout), on Pool.
                    nc.gpsimd.tensor_scalar(
                        msg[:, f0:f1], msg[:, f0:f1], 1.0 / D, 1e-6,
                        op0=mybir.AluOpType.mult, op1=mybir.AluOpType.add,
                    )
                    rstdu = rstdg[:, f0:f1].bitcast(mybir.dt.uint32)
                    nc.gpsimd.tensor_scalar(
                        rstdu, msg[:, f0:f1].bitcast(mybir.dt.uint32), 1, None,
                        op0=mybir.AluOpType.logical_shift_right,
                    )
                    nc.gpsimd.tensor_sub(
                        rstdu,
                        rsq_k[:].broadcast_to([P, nf]).bitcast(mybir.dt.uint32),
                        rstdu,
                    )
                    for _ in range(2):
                        nc.gpsimd.tensor_mul(
                            nwt[:, f0:f1], msg[:, f0:f1], rstdg[:, f0:f1]
                        )
                        nc.gpsimd.tensor_mul(
                            nwt[:, f0:f1], nwt[:, f0:f1], rstdg[:, f0:f1]
                        )
                        nc.gpsimd.tensor_scalar(
                            nwt[:, f0:f1], nwt[:, f0:f1], -0.5, 1.5,
                            op0=mybir.AluOpType.mult, op1=mybir.AluOpType.add,
                        )
                        nc.gpsimd.tensor_mul(
                            rstdg[:, f0:f1], rstdg[:, f0:f1], nwt[:, f0:f1]
                        )
                    ve_.tensor_mul(
                        qkn[:, g0:g1_, :].rearrange("p g (h d) -> p (g h) d", d=D),
                        qkr[:, g0:g1_, :].rearrange("p g (h d) -> p (g h) d", d=D),
                        rstdg[:, f0:f1].unsqueeze(2).broadcast_to([P, nf, D]),
                    )
                # gate r = sigmoid(z) = 0.5 + 0.5*tanh(z/2); csv holds 3*ve.
                rgg = small.tile([P, GRP], f32, tag="rgg")
                nc.vector.tensor_scalar(
                    rgg[:], tgg[:], 0.5, 0.5,
                    op0=mybir.AluOpType.mult, op1=mybir.AluOpType.add,
                )
                vtg = small.tile([P, GRP, D], bf16, tag="vtg", bufs=2)
                nc.gpsimd.tensor_mul(
                    vtg[:],
                    csv_sb[:, bi * GRP : (bi + 1) * GRP, 64:128],
                    rgg[:].unsqueeze(2).broadcast_to([P, GRP, D]),
                )
                nc.gpsimd.tensor_add(
                    v_aug[:, bi * GRP : (bi + 1) * GRP, 0:D],
                    pjg[:, :, 320:384],
                    vtg[:],
                )
                rope.qkn = qkn

            def phase1b(bi):
                qkn = rope.qkn
                for tl in range(GRP):
                    tc_ = bi * GRP + tl
                    tp = psmm.tile([P, 1024], bf16, tag="mm")
                    nc.tensor.transpose(tp[:, 0:P], qkn[:, tl, 0:128], ident[:])
                    nc.tensor.transpose(
                        tp[:, P : 2 * P], qkn[:, tl, 128:256], ident[:]
                    )
                    nc.tensor.transpose(
                        tp[0:D, 2 * P : 3 * P], qkn[:, tl, 256:320], ident[:]
                    )
                    nc.vector.tensor_copy(
                        qT[:, :, tc_ * P : (tc_ + 1) * P],
                        tp[:, 0 : 2 * P].rearrange("p (g t) -> p g t", g=2),
                    )
                    nc.vector.tensor_copy(
                        kT2[0:D, tc_ * P : (tc_ + 1) * P], tp[0:D, 2 * P : 3 * P]
                    )
                    nc.vector.tensor_copy(
                        kT2[D:P, tc_ * P : (tc_ + 1) * P], tp[0:D, 2 * P : 3 * P]
                    )

            def norm3a(bi, pair):
                """denominator reciprocal + bf16 copy + partition broadcast."""
                rst = rs1 if pair == 0 else rs2
                rct = rc1 if pair == 0 else rc2
                rrt = rr1 if pair == 0 else rr2
                nc.vector.reciprocal_approx_fast(
                    rct[:, bi * IB : (bi + 1) * IB],
                    rst[:, bi * IB : (bi + 1) * IB],
                )
                nc.gpsimd.tensor_copy(
                    rrt[:, bi * IB : (bi + 1) * IB],
                    rct[:, bi * IB : (bi + 1) * IB],
                )
                rbp = psy.tile([P, 512], f32, tag="y")
                nc.tensor.matmul(
                    rbp[:],
                    sel[:],
                    rrt[:, bi * IB : (bi + 1) * IB],
                    start=True,
                    stop=True,
                )
                ytp = yT1 if pair == 0 else yT2
                nc.vector.tensor_mul(
                    ytp[:, bi * IB : (bi + 1) * IB],
                    ytp[:, bi * IB : (bi + 1) * IB],
                    rbp[:],
                )

            def norm3b(bi):
                """row-sharded Wo on the normalized yT + bf16 store."""
                for th in range(2):  # token-chunk pairs
                    ob = outsb.tile([P, 2, C], bf16, tag="ob")
                    for tsub in range(2):
                        tc_ = bi * GRP + th * 2 + tsub
                        for cb in range(2):
                            po = psmm.tile([P, 512], f32, tag="mm")
                            nc.tensor.matmul(
                                po[:],
                                yT1[:, tc_ * P : (tc_ + 1) * P],
                                wo_sb[:, 0, cb * 512 : (cb + 1) * 512],
                                start=True,
                                stop=False,
                            )
                            nc.tensor.matmul(
                                po[:],
                                yT2[:, tc_ * P : (tc_ + 1) * P],
                                wo_sb[:, 1, cb * 512 : (cb + 1) * 512],
                                start=False,
                                stop=True,
                            )
                            if cb == 0:
                                nc.vector.tensor_copy(
                                    ob[:, tsub, cb * 512 : (cb + 1) * 512], po[:]
                                )
                            else:
                                nc.scalar.copy(
                                    ob[:, tsub, cb * 512 : (cb + 1) * 512], po[:]
                                )
                    t0 = (bi * GRP + th * 2) * P
                    nc.sync.dma_start(
                        out[t0 : t0 + 2 * P, :],
                        ob[:].rearrange("p a c -> p (a c)"),
                    )

            exp_rr = [0]  # round-robin counter for the DVE exp split

            def phase2(bi, tail=False):
                njt = GRP * (bi + 1)
                for h in range(HQ):
                    yp = psy.tile([D + 1, 512], f32, tag="y")
                    rr = D * (h % 2)
                    qTh = qT[rr : rr + D, h // 2, :]

                    def emit_pair(pt):
                        jt0 = 2 * pt
                        lo0 = max(jt0 - GRP * bi, 0) * P
                        lo1 = max(jt0 + 1 - GRP * bi, 0) * P
                        sp = pssc.tile([P, 2, 512], f32, tag="sc", name="sp")
                        ex = exps.tile([P, 2, 512], bf16, tag="ex", name="ex")
                        for j, (jt, lo) in enumerate(
                            ((jt0, lo0), (jt0 + 1, lo1))
                        ):
                            diag = jt - GRP * bi >= 0
                            nc.tensor.matmul(
                                sp[:, j, lo:512],
                                kT2[rr : rr + D, jt * P : (jt + 1) * P],
                                qTh[:, bi * IB + lo : (bi + 1) * IB],
                                start=True,
                                stop=not diag,
                            )
                            if diag:
                                nc.tensor.matmul(
                                    sp[:, j, lo : lo + P],
                                    ident[:],
                                    mneg[:],
                                    start=False,
                                    stop=True,
                                    skip_group_check=True,
                                )
                        any_diag = jt0 + 1 - GRP * bi >= 0
                        if not any_diag:
                            exp_rr[0] += 1
                        if not any_diag and EXP_SPLIT and exp_rr[0] % EXP_SPLIT == 0:
                            # Schraudolph exp on DVE: bf16 bits via int16
                            nc.vector.tensor_scalar(
                                ex[:, :, lo0:512].bitcast(mybir.dt.int16),
                                sp[:, :, lo0:512],
                                EXP_A,
                                EXP_B,
                                op0=mybir.AluOpType.mult,
                                op1=mybir.AluOpType.add,
                            )
                        else:
                            nc.scalar.activation(
                                ex[:, :, lo0:512],
                                sp[:, :, lo0:512],
                                AF.Exp,
                                scale=SC,
                                bias=zero_sb[:],
                            )
                        return ex, lo0, lo1

                    def emit_av(pt, ex, lo0, lo1):
                        for j, (jt, lo) in enumerate(
                            ((2 * pt, lo0), (2 * pt + 1, lo1))
                        ):
                            nc.tensor.matmul(
                                yp[:, lo:512],
                                v_aug[:, jt, :],
                                ex[:, j, lo:512],
                                start=(jt == 0),
                                stop=(jt == njt - 1),
                            )

                    pending = []
                    for pt in range(njt // 2):
                        pending.append((pt, *emit_pair(pt)))
                        if len(pending) > 1:
                            emit_av(*pending.pop(0))
                    for it in pending:
                        emit_av(*it)
                    rst = rs1 if h < 2 else rs2
                    rrow = 32 * (h % 2)
                    nc.vector.tensor_copy(
                        rst[rrow : rrow + 1, bi * IB : (bi + 1) * IB],
                        yp[D : D + 1, :],
                    )
                    ytp = yT1 if h < 2 else yT2
                    row = D * (h % 2)
                    nc.vector.tensor_copy(
                        ytp[row : row + D, bi * IB : (bi + 1) * IB], yp[0:D, :]
                    )
                    if tail and h % 2 == 1:
                        norm3a(bi, h // 2)

            # ---- pipeline ----
            proj(0, xt0, kc_outer=True)
            tg0 = proj.tgg
            xts = {1: load_x(1)}
            proj(1, xts[1])
            tg1 = proj.tgg
            xts[2] = load_x(2)
            proj(2, xts[2])
            tg2 = proj.tgg
            nc.sync.dma_start(wo_sb[:], woh[:])
            rope(0, tg0)
            phase1b(0)
            rope(1, tg1)
            phase2(0)
            phase1b(1)
            norm3a(0, 0)
            norm3a(0, 1)
            xts[3] = load_x(3)
            proj(3, xts[3])
            tg3 = proj.tgg
            norm3b(0)
            rope(2, tg2)
            phase2(1)
            phase1b(2)
            norm3a(1, 0)
            norm3a(1, 1)
            norm3b(1)
            rope(3, tg3)
            phase2(2)
            phase1b(3)
            norm3a(2, 0)
            norm3a(2, 1)
            norm3b(2)
            phase2(3, tail=True)
            norm3b(3)
    nc.compile()
    return nc


def make_core_inputs(x, ve, cos, sin, Wq, Wk, Wv, Wo, Wg):
    """Slice full inputs into the 8 per-core input maps (b-major, then group).

    All arrays are packed partition-major host-side so each resident load is
    a single contiguous-run DMA.
    """
    import ml_dtypes

    bf = ml_dtypes.bfloat16
    cosf = np.asarray(cos[0, :, 0, :], dtype=np.float32)  # [T, 32]
    sinf = np.asarray(sin[0, :, 0, :], dtype=np.float32)
    in_maps = []
    for c in range(8):
        b, g = c // N_KV_HEAD, c % N_KV_HEAD
        # x packed [P, NBI, KC, IB]
        xhc = np.ascontiguousarray(
            x[b].reshape(NBI, IB, KC, P).transpose(3, 0, 2, 1)
        ).astype(bf)
        wq = Wq[g * 256 : (g + 1) * 256, :]           # [256, C]
        wk = Wk[g * D : (g + 1) * D, :]               # [64, C]
        wv = Wv[g * D : (g + 1) * D, :]
        wg_col = np.zeros((C, 1), np.float32)
        wg_col[:12, 0] = Wg[g]
        wrc = np.concatenate([wq.T, wk.T, wv.T, wg_col], axis=1)  # [C, WJ]
        wrhc = np.ascontiguousarray(
            wrc.reshape(KC, P, WJ).transpose(1, 0, 2)
        ).astype(bf)                                  # [P, KC, WJ]
        csv = np.concatenate(
            [cosf, sinf, 3.0 * ve[b, :, g * D : (g + 1) * D]], axis=1
        )                                             # [T, 128]
        csvc = np.ascontiguousarray(
            csv.reshape(NT, P, 128).transpose(1, 0, 2).reshape(P, NT * 128)
        ).astype(bf)
        woTc = Wo[:, g * 256 : (g + 1) * 256].T       # [256, C]
        wohc = np.ascontiguousarray(
            woTc.reshape(2, P, C).transpose(1, 0, 2)
        ).astype(bf)                                  # [P, 2, C]
        in_maps.append({"xh": xhc, "wrh": wrhc, "csvh": csvc, "woh": wohc})
    return in_maps


_PROGRAM = None


def kernel(x, ve, cos, sin, Wq, Wk, Wv, Wo, Wg, _trace=False):
    from concourse.bass_utils import run_bass_kernel_spmd

    # coerce to host fp32 ndarrays up front (harness may pass jax arrays)
    x, ve, cos, sin, Wq, Wk, Wv, Wo, Wg = (
        np.asarray(a, dtype=np.float32)
        for a in (x, ve, cos, sin, Wq, Wk, Wv, Wo, Wg)
    )
    global _PROGRAM
    if _PROGRAM is None:
        _PROGRAM = build_program()
    nc = _PROGRAM
    in_maps = make_core_inputs(x, ve, cos, sin, Wq, Wk, Wv, Wo, Wg)
    res = run_bass_kernel_spmd(nc, in_maps, list(range(8)), trace=_trace)
    outs = [r["out"] for r in res.results]
    full = np.zeros((B, T, C), np.float32)
    for c in range(8):
        full[c // N_KV_HEAD] += np.asarray(outs[c], dtype=np.float32)
    if _trace:
        kernel.last_results = res
    return full


# revision 44
# speedup vs baseline: 1.1757x; 1.1757x over previous
"""Trainium2 Bass kernel for nn_Attention_5299989643989.

GQA attention forward (B=2, T=2048, C=1024, 16 q heads / 4 kv heads, D=64)
with value-embedding gating, rotary embedding, qk rms-norm, causal softmax.

Sharding: 8 cores = batch (2) x kv-head-group (4).  Each core computes its
4 q heads / 1 kv head end-to-end plus the Wo row-shard partial output; the
host sums the 4 partials per batch (the Wo all-reduce, done at unshard).

bf16 datapath: inputs are packed host-side into partition-major bf16 arrays
so each resident load is one large DMA; all matmuls run in bf16 (1 cycle/row
on the PE at any width); the output is stored bf16 and the host accumulates
partials in f32.

Schedule notes (cost-model driven):
 - The PE p-states reward continuous execution, so projections run 2-3
   query-groups ahead of the attention loop and the first group's
   projection runs kc-outer across four open PSUM banks so matmuls start
   as soon as the first interleaved weight/x DMA slice lands.
 - The causal mask is applied on the PE: diagonal score tiles accumulate a
   -1e9 strict-lower-triangular constant before the exp, so no vector
   engine sits between the exp and the AV matmul.
 - exp is batched two PSUM banks per activation call ([P, 2, 512-lo] APs)
   to amortize the ACT access latency.
 - GPSIMD has no PSUM port: all PSUM evacuation is on DVE (pjg, qT, kT2,
   denominators, yT, half the output tiles) and ACT (other half of the
   output tiles); Pool does SBUF-only work (rsqrt Newton chain, ve-gating,
   reciprocal->bf16 copies, constants).
"""

import numpy as np

import concourse.bacc as bacc
import concourse.bass as bass
import concourse.tile as tile
from concourse import mybir
from concourse.masks import make_identity, make_upper_triangular

f32 = mybir.dt.float32
bf16 = mybir.dt.bfloat16
AF = mybir.ActivationFunctionType

B, T, C = 2, 2048, 1024
N_HEAD, N_KV_HEAD, D = 16, 4, 64
HQ = N_HEAD // N_KV_HEAD  # q heads per core = 4
P = 128
NT = T // P       # 16 token chunks
KC = C // P       # 8 contraction chunks
IB = 512          # query block
NBI = T // IB     # 4 query blocks
GRP = IB // P     # 4 token chunks per query block
SC = 1.2 * 1.2 / 8.0  # folded qk scale: rms 1.2 factors * 1/sqrt(64)
H32 = D // 2
WJ = 385          # jammed projection width: q 256 | k 64 | v 64 | gate 1

# Schraudolph exp on DVE (bf16 bit pattern via int16 convert):
#   exp(SC*s) ~= bitcast_bf16(int16(EXP_A*s + EXP_B))
# EXP_A = SC * 2^7/ln2; EXP_B = 127*2^7 - 2^7*C with C ~= 0.0579 ln-units
# (minimum-RMS magic constant, Schraudolph 1999).  Only used on
# off-diagonal score tiles where |s| <= 64 so the int16 never saturates.
EXP_A = SC * 184.66496  # SC * 2^7 / ln 2
EXP_B = 16248.58        # (127*2^23 - 486411) / 2^16, min-RMS Schraudolph
EXP_SPLIT = 3     # every EXP_SPLIT'th off-diagonal exp pair runs on DVE
ROPE_POOL = False  # rope+normalize for groups >= 1 run on Pool


def build_program():
    nc = bacc.Bacc("TRN2", target_bir_lowering=False, debug=False, num_devices=8)

    xh = nc.dram_tensor("xh", [P, NBI, KC, IB], bf16, kind="ExternalInput")
    wrh = nc.dram_tensor("wrh", [P, KC, WJ], bf16, kind="ExternalInput")
    csvh = nc.dram_tensor("csvh", [P, NT * 128], bf16, kind="ExternalInput")
    woh = nc.dram_tensor("woh", [P, 2, C], bf16, kind="ExternalInput")
    out = nc.dram_tensor("out", [T, C], bf16, kind="ExternalOutput")

    with tile.TileContext(nc) as tc:
        with (
            tc.tile_pool(name="consts", bufs=1) as consts,
            tc.tile_pool(name="resid", bufs=1) as resid,
            tc.tile_pool(name="xload", bufs=2) as xload,
            tc.tile_pool(name="rot", bufs=2) as rot,
            tc.tile_pool(name="small", bufs=4) as small,
            tc.tile_pool(name="exps", bufs=6) as exps,
            tc.tile_pool(name="outsb", bufs=2) as outsb,
            tc.tile_pool(name="psmm", bufs=2, space="PSUM") as psmm,
            tc.tile_pool(name="pssc", bufs=2, space="PSUM") as pssc,
            tc.tile_pool(name="psy", bufs=2, space="PSUM") as psy,
        ):
            # ---- group-0 resident loads.  Each DMA lands in its own tile
            # so the first projection matmuls only wait for their own slice
            # (the tile framework merges waits per tile, not per region). ----
            wr_a = consts.tile([P, 4, WJ], bf16)
            wr_b = consts.tile([P, 4, WJ], bf16)
            xt0p = []
            nc.sync.dma_start(wr_a[:], wrh[:, 0:4, :])
            for pr in range(4):
                x0 = xload.tile([P, 2, IB], bf16, name=f"xt0p{pr}", tag=f"x0{pr}")
                xt0p.append(x0)
            nc.sync.dma_start(xt0p[0][:], xh[:, 0, 0:2, :])
            nc.sync.dma_start(xt0p[1][:], xh[:, 0, 2:4, :])
            # csv = cos(32) | sin(32) | 3*ve (64) per token; early, it gates
            # the first rope chain
            csv_sb = consts.tile([P, NT, 128], bf16)
            nc.sync.dma_start(
                csv_sb[:], csvh[:].rearrange("p (n d) -> p n d", d=128)
            )
            nc.sync.dma_start(wr_b[:], wrh[:, 4:8, :])
            nc.sync.dma_start(xt0p[2][:], xh[:, 0, 4:6, :])
            nc.sync.dma_start(xt0p[3][:], xh[:, 0, 6:8, :])
            ident = consts.tile([P, P], bf16)
            make_identity(nc, ident[:])
            tri_sb = consts.tile([P, P], bf16)
            make_upper_triangular(nc, tri_sb[:], val=1.0, diag=True)
            # selector for the denominator broadcast: rows 0/32 of rr ->
            # partition halves 0:64 / 64:128 of the outer-product result
            sel = consts.tile([33, P], bf16)
            nc.gpsimd.memset(sel[:], 0.0)
            nc.gpsimd.memset(sel[0:1, 0:64], 1.0)
            nc.gpsimd.memset(sel[32:33, 64:128], 1.0)
            zero_sb = consts.tile([P, 1], f32)
            nc.vector.memset(zero_sb[:], 0.0)
            rsq_k = consts.tile([P, 1], mybir.dt.uint32)
            nc.vector.memset(rsq_k[:], 0x5F3759DF)

            # ---- residents written by the kernel ----
            qT = resid.tile([P, 2, T], bf16)   # [h0|h1] rows, [h2|h3] rows
            kT2 = resid.tile([P, T], bf16)     # kT duplicated in both halves
            v_aug = resid.tile([P, NT, D + 1], bf16)  # v plus ones column
            nc.gpsimd.memset(v_aug[:, :, D : D + 1], 1.0)
            yT1 = resid.tile([P, T], bf16)        # yT heads 0,1
            yT2 = resid.tile([P, T], bf16)        # yT heads 2,3
            rs1 = resid.tile([33, T], f32)   # denominators: h0 row0, h1 row32
            rs2 = resid.tile([33, T], f32)
            nc.gpsimd.memset(rs1[:], 1.0)
            nc.gpsimd.memset(rs2[:], 1.0)
            rc1 = resid.tile([33, T], f32)
            rc2 = resid.tile([33, T], f32)
            rr1 = resid.tile([33, T], bf16)   # bf16 copies for the PE
            rr2 = resid.tile([33, T], bf16)

            wo_sb = consts.tile([P, 2, C], bf16)

            def load_x(bi):
                xt = xload.tile([P, KC, IB], bf16, name=f"xt{bi}", tag="xt")
                nc.sync.dma_start(xt[:], xh[:, bi])
                return xt

            pjgs = {}
            tggs = {}
            qkns = {}

            def proj_tail(bi, pjg):
                # gate tanh for the whole group in one ACT call
                tgg = small.tile([P, GRP], f32, tag="tgg")
                nc.scalar.activation(
                    tgg[:],
                    pjg[:, :, 384:385].rearrange("p g o -> p (g o)"),
                    AF.Tanh,
                    scale=0.5,
                    bias=zero_sb[:],
                )
                tggs[bi] = tgg

            def wr_ap(kc):
                return (wr_a if kc < 4 else wr_b)[:, kc % 4, :]

            def proj0():
                """Group-0 projection, contraction outermost across four open
                PSUM banks to chase the interleaved weight/x DMAs."""
                pjg = rot.tile([P, GRP, WJ], bf16, tag="pjg", bufs=3)
                pjgs[0] = pjg
                pj_aps = []
                for tl in range(GRP):
                    if tl < 2:
                        pjt = psmm.tile([P, 512], f32, tag="mm", name=f"pjt{tl}")
                        pj_aps.append(pjt[:, 0:WJ])
                    else:
                        pjt = pssc.tile(
                            [P, 2, 512], f32, tag="sc", name=f"pjt{tl}"
                        )
                        pj_aps.append(pjt[:, 0, 0:WJ])
                for kc in range(KC):
                    for tl in range(GRP):
                        nc.tensor.matmul(
                            pj_aps[tl],
                            xt0p[kc // 2][:, kc % 2, tl * P : (tl + 1) * P],
                            wr_ap(kc),
                            start=(kc == 0),
                            stop=(kc == KC - 1),
                        )
                for tl in range(GRP):
                    nc.scalar.copy(pjg[:, tl, :], pj_aps[tl])
                proj_tail(0, pjg)

            def proj_gen(bi, xt, copy_eng=None):
                """Jammed projection for group bi; yields between matmuls so
                phase2 can weave these into the PE stream as fillers."""
                pjg = rot.tile([P, GRP, WJ], bf16, tag="pjg", bufs=3)
                pjgs[bi] = pjg
                for tl in range(GRP):
                    pj = psmm.tile([P, 512], f32, tag="mm")
                    for kc in range(KC):
                        nc.tensor.matmul(
                            pj[:, 0:WJ],
                            xt[:, kc, tl * P : (tl + 1) * P],
                            wr_ap(kc),
                            start=(kc == 0),
                            stop=(kc == KC - 1),
                        )
                        yield
                    if copy_eng is None:
                        nc.scalar.copy(pjg[:, tl, :], pj[:, 0:WJ])
                    else:
                        copy_eng.tensor_copy(pjg[:, tl, :], pj[:, 0:WJ])
                proj_tail(bi, pjg)

            def rope(bi, halves=2):
                """rope + rms-norm + rstd + normalize + ve-gating for group bi."""
                tgg = tggs[bi]
                pjg = pjgs[bi]
                qkr = rot.tile([P, GRP, 320], bf16, tag="qkr", bufs=2)
                tmp = rot.tile([P, GRP, 160], bf16, tag="tmp", bufs=2)
                sqg = rot.tile([P, GRP, 320], bf16, tag="sqg", bufs=2)
                msg = small.tile([P, GRP * 5], f32, tag="msg")
                rstdg = small.tile([P, GRP * 5], f32, tag="rstdg")
                nwt = small.tile([P, GRP * 5], f32, tag="nwt")
                qkn = rot.tile([P, GRP, 320], bf16, tag="qkn", bufs=2)
                ve_ = nc.gpsimd if (ROPE_POOL and bi >= 2) else nc.vector
                gstep = GRP // halves
                for hf in range(halves):
                    g0, g1_ = gstep * hf, gstep * (hf + 1)
                    f0, f1 = 5 * gstep * hf, 5 * gstep * (hf + 1)
                    nf = f1 - f0
                    qv5 = pjg[:, g0:g1_, 0:320].rearrange(
                        "p g (h d) -> p g h d", d=D
                    )
                    ro5 = qkr[:, g0:g1_, :].rearrange("p g (h d) -> p g h d", d=D)
                    t5 = tmp[:, g0:g1_, :].rearrange("p g (h d) -> p g h d", d=H32)
                    cs = csv_sb[:, bi * GRP + g0 : bi * GRP + g1_, 0:32]
                    sn = csv_sb[:, bi * GRP + g0 : bi * GRP + g1_, 32:64]
                    cos5 = cs.unsqueeze(2).broadcast_to([P, g1_ - g0, 5, H32])
                    sin5 = sn.unsqueeze(2).broadcast_to([P, g1_ - g0, 5, H32])
                    q1 = qv5[:, :, :, 0:H32]
                    q2 = qv5[:, :, :, H32:D]
                    ve_.tensor_mul(ro5[:, :, :, 0:H32], q1, cos5)
                    ve_.tensor_mul(t5[:], q2, sin5)
                    ve_.tensor_add(
                        ro5[:, :, :, 0:H32], ro5[:, :, :, 0:H32], t5[:]
                    )
                    ve_.tensor_mul(ro5[:, :, :, H32:D], q2, cos5)
                    ve_.tensor_mul(t5[:], q1, sin5)
                    ve_.tensor_sub(
                        ro5[:, :, :, H32:D], ro5[:, :, :, H32:D], t5[:]
                    )

                    nc.vector.tensor_mul(
                        sqg[:, g0:g1_, :], qkr[:, g0:g1_, :], qkr[:, g0:g1_, :]
                    )
                    nc.vector.reduce_sum(
                        msg[:, f0:f1],
                        sqg[:, g0:g1_, :].rearrange("p g (h d) -> p (g h) d", d=D),
                        axis=mybir.AxisListType.X,
                    )
                    # m = mean + eps; rstd = m^-1/2 by bit-trick seed + two
                    # Newton iterations (f32 bit layout), on Pool.
                    nc.vector.tensor_scalar(
                        msg[:, f0:f1], msg[:, f0:f1], 1.0 / D, 1e-6,
                        op0=mybir.AluOpType.mult, op1=mybir.AluOpType.add,
                    )
                    rstdu = rstdg[:, f0:f1].bitcast(mybir.dt.uint32)
                    nc.vector.tensor_scalar(
                        rstdu, msg[:, f0:f1].bitcast(mybir.dt.uint32), 1, None,
                        op0=mybir.AluOpType.logical_shift_right,
                    )
                    nc.vector.tensor_sub(
                        rstdu,
                        rsq_k[:].broadcast_to([P, nf]).bitcast(mybir.dt.uint32),
                        rstdu,
                    )
                    for _ in range(2):
                        nc.vector.tensor_mul(
                            nwt[:, f0:f1], msg[:, f0:f1], rstdg[:, f0:f1]
                        )
                        nc.vector.tensor_mul(
                            nwt[:, f0:f1], nwt[:, f0:f1], rstdg[:, f0:f1]
                        )
                        nc.vector.tensor_scalar(
                            nwt[:, f0:f1], nwt[:, f0:f1], -0.5, 1.5,
                            op0=mybir.AluOpType.mult, op1=mybir.AluOpType.add,
                        )
                        nc.vector.tensor_mul(
                            rstdg[:, f0:f1], rstdg[:, f0:f1], nwt[:, f0:f1]
                        )
                    nrm_ = nc.vector if bi == 0 else nc.gpsimd
                    nrm_.tensor_mul(
                        qkn[:, g0:g1_, :].rearrange("p g (h d) -> p (g h) d", d=D),
                        qkr[:, g0:g1_, :].rearrange("p g (h d) -> p (g h) d", d=D),
                        rstdg[:, f0:f1].unsqueeze(2).broadcast_to([P, nf, D]),
                    )
                # gate r = sigmoid(z) = 0.5 + 0.5*tanh(z/2); csv holds 3*ve.
                rgg = small.tile([P, GRP], f32, tag="rgg")
                nc.vector.tensor_scalar(
                    rgg[:], tgg[:], 0.5, 0.5,
                    op0=mybir.AluOpType.mult, op1=mybir.AluOpType.add,
                )
                vtg = small.tile([P, GRP, D], bf16, tag="vtg", bufs=2)
                nc.gpsimd.tensor_mul(
                    vtg[:],
                    csv_sb[:, bi * GRP : (bi + 1) * GRP, 64:128],
                    rgg[:].unsqueeze(2).broadcast_to([P, GRP, D]),
                )
                nc.gpsimd.tensor_add(
                    v_aug[:, bi * GRP : (bi + 1) * GRP, 0:D],
                    pjg[:, :, 320:384],
                    vtg[:],
                )
                qkns[bi] = qkn

            def phase1b_gen(bi):
                qkn = qkns[bi]
                for tl in range(GRP):
                    tc_ = bi * GRP + tl
                    tp = psmm.tile([P, 1024], bf16, tag="mm")
                    nc.tensor.transpose(tp[:, 0:P], qkn[:, tl, 0:128], ident[:])
                    nc.tensor.transpose(
                        tp[:, P : 2 * P], qkn[:, tl, 128:256], ident[:]
                    )
                    nc.tensor.transpose(
                        tp[0:D, 2 * P : 3 * P], qkn[:, tl, 256:320], ident[:]
                    )
                    nc.vector.tensor_copy(
                        qT[:, :, tc_ * P : (tc_ + 1) * P],
                        tp[:, 0 : 2 * P].rearrange("p (g t) -> p g t", g=2),
                    )
                    nc.vector.tensor_copy(
                        kT2[0:D, tc_ * P : (tc_ + 1) * P], tp[0:D, 2 * P : 3 * P]
                    )
                    nc.vector.tensor_copy(
                        kT2[D:P, tc_ * P : (tc_ + 1) * P], tp[0:D, 2 * P : 3 * P]
                    )
                    yield

            def norm3a(bi, pair):
                """denominator reciprocal + bf16 copy + partition broadcast."""
                rst = rs1 if pair == 0 else rs2
                rct = rc1 if pair == 0 else rc2
                rrt = rr1 if pair == 0 else rr2
                nc.vector.reciprocal_approx_fast(
                    rct[:, bi * IB : (bi + 1) * IB],
                    rst[:, bi * IB : (bi + 1) * IB],
                )
                nc.gpsimd.tensor_copy(
                    rrt[:, bi * IB : (bi + 1) * IB],
                    rct[:, bi * IB : (bi + 1) * IB],
                )
                rbp = psy.tile([P, 512], f32, tag="y")
                nc.tensor.matmul(
                    rbp[:],
                    sel[:],
                    rrt[:, bi * IB : (bi + 1) * IB],
                    start=True,
                    stop=True,
                )
                ytp = yT1 if pair == 0 else yT2
                nc.vector.tensor_mul(
                    ytp[:, bi * IB : (bi + 1) * IB],
                    ytp[:, bi * IB : (bi + 1) * IB],
                    rbp[:],
                )

            def norm3b_gen(bi):
                """row-sharded Wo on the normalized yT + bf16 store."""
                for th in range(2):  # token-chunk pairs
                    ob = outsb.tile([P, 2, C], bf16, tag="ob")
                    for tsub in range(2):
                        tc_ = bi * GRP + th * 2 + tsub
                        for cb in range(2):
                            po = psmm.tile([P, 512], f32, tag="mm")
                            nc.tensor.matmul(
                                po[:],
                                yT1[:, tc_ * P : (tc_ + 1) * P],
                                wo_sb[:, 0, cb * 512 : (cb + 1) * 512],
                                start=True,
                                stop=False,
                            )
                            nc.tensor.matmul(
                                po[:],
                                yT2[:, tc_ * P : (tc_ + 1) * P],
                                wo_sb[:, 1, cb * 512 : (cb + 1) * 512],
                                start=False,
                                stop=True,
                            )
                            yield
                            if cb == 0:
                                nc.vector.tensor_copy(
                                    ob[:, tsub, cb * 512 : (cb + 1) * 512], po[:]
                                )
                            else:
                                nc.scalar.copy(
                                    ob[:, tsub, cb * 512 : (cb + 1) * 512], po[:]
                                )
                    t0 = (bi * GRP + th * 2) * P
                    nc.sync.dma_start(
                        out[t0 : t0 + 2 * P, :].rearrange("(a t) c -> t a c", a=2),
                        ob[:],
                    )

            exp_rr = [0]  # round-robin counter for the DVE exp split

            def phase2(bi, fillers=None, rate=0.0):
                njt = GRP * (bi + 1)
                acc = [0.0]

                def fill():
                    if fillers is None:
                        return
                    acc[0] += rate
                    while acc[0] >= 1.0:
                        acc[0] -= 1.0
                        if next(fillers, None) is None:
                            acc[0] = 0.0
                            break
                for h in range(HQ):
                    yp = psy.tile([D + 1, 512], f32, tag="y")
                    rr = D * (h % 2)
                    qTh = qT[rr : rr + D, h // 2, :]

                    def emit_pair(pt):
                        jt0 = 2 * pt
                        lo0 = max(jt0 - GRP * bi, 0) * P
                        lo1 = max(jt0 + 1 - GRP * bi, 0) * P
                        sp = pssc.tile([P, 2, 512], f32, tag="sc", name="sp")
                        ex = exps.tile([P, 2, 512], bf16, tag="ex", name="ex")
                        for j, jt in enumerate((jt0, jt0 + 1)):
                            # compute from lo0 (not this bank's lo) so the
                            # exp's [lo0:512] read is fully initialized; the
                            # extra strip is below the tile diagonal and the
                            # AV never reads it
                            nc.tensor.matmul(
                                sp[:, j, lo0:512],
                                kT2[rr : rr + D, jt * P : (jt + 1) * P],
                                qTh[:, bi * IB + lo0 : (bi + 1) * IB],
                                start=True,
                                stop=True,
                            )
                        any_diag = jt0 + 1 - GRP * bi >= 0
                        if not any_diag:
                            exp_rr[0] += 1
                        if (not any_diag and bi >= 2 and EXP_SPLIT
                                and pt < njt // 2 - 2
                                and exp_rr[0] % EXP_SPLIT == 0):
                            # Schraudolph exp on DVE: bf16 bits via int16
                            nc.vector.tensor_scalar(
                                ex[:, :, lo0:512].bitcast(mybir.dt.int16),
                                sp[:, :, lo0:512],
                                EXP_A,
                                EXP_B,
                                op0=mybir.AluOpType.mult,
                                op1=mybir.AluOpType.add,
                            )
                        else:
                            nc.scalar.activation(
                                ex[:, :, lo0:512],
                                sp[:, :, lo0:512],
                                AF.Exp,
                                scale=SC,
                                bias=zero_sb[:],
                            )
                        return ex, lo0, lo1

                    def emit_av(pt, ex, lo0, lo1):
                        for j, (jt, lo) in enumerate(
                            ((2 * pt, lo0), (2 * pt + 1, lo1))
                        ):
                            if jt - GRP * bi >= 0:
                                nc.gpsimd.tensor_mul(
                                    ex[:, j, lo : lo + P],
                                    ex[:, j, lo : lo + P],
                                    tri_sb[:],
                                )
                            nc.tensor.matmul(
                                yp[:, lo:512],
                                v_aug[:, jt, :],
                                ex[:, j, lo:512],
                                start=(jt == 0),
                                stop=(jt == njt - 1),
                            )

                    pending = []
                    for pt in range(njt // 2):
                        pending.append((pt, *emit_pair(pt)))
                        fill()
                        if len(pending) > 4:
                            emit_av(*pending.pop(0))
                    for it in pending:
                        emit_av(*it)
                    rst = rs1 if h < 2 else rs2
                    rrow = 32 * (h % 2)
                    nc.vector.tensor_copy(
                        rst[rrow : rrow + 1, bi * IB : (bi + 1) * IB],
                        yp[D : D + 1, :],
                    )
                    ytp = yT1 if h < 2 else yT2
                    row = D * (h % 2)
                    nc.vector.tensor_copy(
                        ytp[row : row + D, bi * IB : (bi + 1) * IB], yp[0:D, :]
                    )
                    if h % 2 == 1:
                        norm3a(bi, h // 2)
                if fillers is not None:
                    for _ in fillers:
                        pass


            # ---- pipeline ----
            import itertools

            def drain(gen):
                for _ in gen:
                    pass

            proj0()
            xts = {1: load_x(1)}
            drain(proj_gen(1, xts[1]))
            xts[2] = load_x(2)
            xts[3] = load_x(3)
            drain(proj_gen(2, xts[2]))
            nc.sync.dma_start(wo_sb[:], woh[:])
            rope(0)
            drain(phase1b_gen(0))
            rope(1)
            phase2(
                0,
                fillers=itertools.chain(proj_gen(3, xts[3]), phase1b_gen(1)),
                rate=5.5,
            )
            rope(2)
            phase2(
                1,
                fillers=itertools.chain(norm3b_gen(0), phase1b_gen(2)),
                rate=1.25,
            )
            rope(3)
            phase2(
                2,
                fillers=itertools.chain(norm3b_gen(1), phase1b_gen(3)),
                rate=0.85,
            )
            phase2(3, fillers=norm3b_gen(2), rate=0.3)
            drain(norm3b_gen(3))
    nc.compile()
    return nc


def make_core_inputs(x, ve, cos, sin, Wq, Wk, Wv, Wo, Wg):
    """Slice full inputs into the 8 per-core input maps (b-major, then group).

    All arrays are packed partition-major host-side so each resident load is
    a single contiguous-run DMA.
    """
    import ml_dtypes

    bf = ml_dtypes.bfloat16
    cosf = np.asarray(cos[0, :, 0, :], dtype=np.float32)  # [T, 32]
    sinf = np.asarray(sin[0, :, 0, :], dtype=np.float32)
    in_maps = []
    for c in range(8):
        b, g = c // N_KV_HEAD, c % N_KV_HEAD
        # x packed [P, NBI, KC, IB]
        xhc = np.ascontiguousarray(
            x[b].reshape(NBI, IB, KC, P).transpose(3, 0, 2, 1)
        ).astype(bf)
        wq = Wq[g * 256 : (g + 1) * 256, :]           # [256, C]
        wk = Wk[g * D : (g + 1) * D, :]               # [64, C]
        wv = Wv[g * D : (g + 1) * D, :]
        wg_col = np.zeros((C, 1), np.float32)
        wg_col[:12, 0] = Wg[g]
        wrc = np.concatenate([wq.T, wk.T, wv.T, wg_col], axis=1)  # [C, WJ]
        wrhc = np.ascontiguousarray(
            wrc.reshape(KC, P, WJ).transpose(1, 0, 2)
        ).astype(bf)                                  # [P, KC, WJ]
        csv = np.concatenate(
            [cosf, sinf, 3.0 * ve[b, :, g * D : (g + 1) * D]], axis=1
        )                                             # [T, 128]
        csvc = np.ascontiguousarray(
            csv.reshape(NT, P, 128).transpose(1, 0, 2).reshape(P, NT * 128)
        ).astype(bf)
        woTc = Wo[:, g * 256 : (g + 1) * 256].T       # [256, C]
        wohc = np.ascontiguousarray(
            woTc.reshape(2, P, C).transpose(1, 0, 2)
        ).astype(bf)                                  # [P, 2, C]
        in_maps.append({"xh": xhc, "wrh": wrhc, "csvh": csvc, "woh": wohc})
    return in_maps


_PROGRAM = None


def kernel(x, ve, cos, sin, Wq, Wk, Wv, Wo, Wg, _trace=False):
    from concourse.bass_utils import run_bass_kernel_spmd

    # coerce to host fp32 ndarrays up front (harness may pass jax arrays)
    x, ve, cos, sin, Wq, Wk, Wv, Wo, Wg = (
        np.asarray(a, dtype=np.float32)
        for a in (x, ve, cos, sin, Wq, Wk, Wv, Wo, Wg)
    )
    global _PROGRAM
    if _PROGRAM is None:
        _PROGRAM = build_program()
    nc = _PROGRAM
    in_maps = make_core_inputs(x, ve, cos, sin, Wq, Wk, Wv, Wo, Wg)
    res = run_bass_kernel_spmd(nc, in_maps, list(range(8)), trace=_trace)
    outs = [r["out"] for r in res.results]
    full = np.zeros((B, T, C), np.float32)
    for c in range(8):
        full[c // N_KV_HEAD] += np.asarray(outs[c], dtype=np.float32)
    if _trace:
        kernel.last_results = res
    return full
